# revision 23
# baseline (speedup 1.0000x reference)
"""EventAugmentedLSTMCell fused kernel for 8 Trainium2 NeuronCores.

Data-parallel over batch: each of the 8 cores processes 512 of the 4096
batch rows.  Host prepares transposed inputs/weights; device does all
B-proportional math:
  det  = x @ det_w.T            (PE)   -> mask = det > -det_b
  v    = x @ value_w.T (+b)     (PE)
  fused= sum_s w[s]*slots[b,s,:] + mask*w[ptr]*(v - slots[b,ptr,:])
         (+ sum_s w[s]*pos_emb[s] folded into proj bias on host)
  h_mem= fused @ proj_w.T + proj_b_eff                    (PE)
  gates= [x, h_mem, h] @ [W_ih|W_hh].T (+b)               (PE)
  LSTM elementwise                                        (ACT/DVE)
  slots_new = slots, with rows [b, ptr[b]] patched to v where mask
         (bulk passthrough SBUF copy + indirect-DMA scatter patch)
  ptr_new = (ptr + mask) % S

Scheduling notes:
 - the x/h contributions to the gates matmul are issued before the slots
   weighted-sum so only the h_mem contribution sits on the critical tail
 - slots weighted-sum MACs are split across DVE and GPSIMD (two partial
   accumulators) to shorten the per-tile reduction
 - loads go on the sync (SP) HWDGE ring, stores on the scalar (ACT) ring
   to avoid head-of-line blocking between them
"""

import numpy as np

B, D, H, S = 4096, 512, 512, 32
M = 8            # cores
BL = B // M      # 512 batch rows per core
P = 128          # partitions
T = BL // P      # 4 batch tiles per core
SJ = 4           # s-values per slots chunk
SC = S // SJ     # 8 slots chunks per batch tile

_NC_CACHE = {}
TRACE = False
MM_F32R = True
LAST_EXEC_NS = None
LAST_RESULTS = None


def _build_nc(cfg):
    import concourse.bass as bass
    import concourse.mybir as mybir
    from concourse.bacc import Bacc
    from concourse.tile import TileContext
    from concourse.masks import make_identity
    from bass_rust import add_dep_helper

    fp32 = mybir.dt.float32
    i32 = mybir.dt.int32
    Alu = mybir.AluOpType
    Act = mybir.ActivationFunctionType

    w_s = cfg["w"]                    # tuple of 32 softmax weights
    neg_det_b = cfg["neg_det_b"]
    has_gbias = cfg["has_gbias"]
    has_vbias = cfg["has_vbias"]
    mm_f32r = cfg.get("mm_f32r", False)
    uniform_w = all(x == w_s[0] for x in w_s)

    nc = Bacc()
    # float32r tensors are stored as plain f32 bytes; the PE consumes them
    # through its fast (4x) fp32 path with reduced internal precision.
    mmdt = mybir.dt.float32r if mm_f32r else fp32

    # ---- DRAM I/O ----
    xT_d = nc.dram_tensor("xT", [D, BL], fp32, kind="ExternalInput")
    if mm_f32r:
        # same bytes as xT, dtype-tagged float32r for the gates matmul
        xTr_d = nc.dram_tensor("xTr", [D, BL], mmdt, kind="ExternalInput")
    hT_d = nc.dram_tensor("hT", [H, BL], mmdt, kind="ExternalInput")
    c_d = nc.dram_tensor("c", [BL, H], fp32, kind="ExternalInput")
    slots_d = nc.dram_tensor("slots", [BL * S, D], fp32, kind="ExternalInput")
    ptrf_d = nc.dram_tensor("ptrf", [BL, 1], fp32, kind="ExternalInput")
    idx_d = nc.dram_tensor("idx", [BL, 1], i32, kind="ExternalInput")
    wptr_d = nc.dram_tensor("wptr", [BL, 1], fp32, kind="ExternalInput")
    wgT_d = nc.dram_tensor("wgT", [D + H + H, 4 * H], mmdt, kind="ExternalInput")
    vwT_d = nc.dram_tensor("vwT", [D, D], fp32, kind="ExternalInput")
    dtT_d = nc.dram_tensor("dtT", [D, 1], fp32, kind="ExternalInput")
    gbrow_d = nc.dram_tensor("gbrow", [1, 4 * H], mmdt, kind="ExternalInput")
    ones_d = nc.dram_tensor("onesrow", [1, P], mmdt, kind="ExternalInput")
    if has_vbias:
        biasv_d = nc.dram_tensor("biasv", [P, D], fp32, kind="ExternalInput")

    h_o = nc.dram_tensor("h_out", [BL, H], fp32, kind="ExternalOutput")
    c_o = nc.dram_tensor("c_out", [BL, H], fp32, kind="ExternalOutput")
    slots_o = nc.dram_tensor("slots_out", [BL * S, D], fp32, kind="ExternalOutput")
    ptr_o = nc.dram_tensor("ptr_out", [BL, 1], i32, kind="ExternalOutput")

    KG = (D + H + H) // P             # 12 contraction chunks for gates

    # engine split for the 32 weighted-sum accumulate ops (DVE ~2x GPSIMD
    # throughput).  GPSIMD can only run plain TensorTensor/copy, so it only
    # participates when the softmax weights are uniform (adds, scale folded
    # in at the merge).
    mac_on_gp = [uniform_w and (s % 3 == 2) for s in range(S)]

    with TileContext(nc) as tc:
        with (
            tc.tile_pool(name="const", bufs=1) as cpool,
            tc.tile_pool(name="work", bufs=2) as wpool,
            tc.tile_pool(name="slots", bufs=4) as spool,
            tc.tile_pool(name="psA", bufs=1, space="PSUM") as psA,
            tc.tile_pool(name="psG", bufs=1, space="PSUM") as psG,
        ):
            # ---- small constants needed by the earliest compute ----
            ident = cpool.tile([P, P], fp32)
            make_identity(nc, ident[:])

            vw_s = cpool.tile([P, 4, D], fp32)
            nc.sync.dma_start(
                out=vw_s[:], in_=vwT_d[:].rearrange("(k p) n -> p k n", p=P)
            )
            dt_s = cpool.tile([P, 4, 1], fp32)
            nc.sync.dma_start(
                out=dt_s[:], in_=dtT_d[:].rearrange("(k p) n -> p k n", p=P)
            )
            ones_t = cpool.tile([1, P], mmdt)
            nc.sync.dma_start(out=ones_t[:], in_=ones_d[:])
            gb_s = cpool.tile([1, 4 * H], mmdt)
            nc.sync.dma_start(out=gb_s[:], in_=gbrow_d[:])
            # declared now, loaded after the first tile's input DMAs
            wg_s = cpool.tile([P, KG, 4 * H], mmdt)
            if has_vbias:
                bv_s = cpool.tile([P, D], fp32)

            slots3 = slots_d[:].rearrange("(b s) d -> b s d", s=S)
            slots3o = slots_o[:].rearrange("(b s) d -> b s d", s=S)
            xT4 = xT_d[:].rearrange("(k p) b -> p k b", p=P)
            hT4 = hT_d[:].rearrange("(k p) b -> p k b", p=P)
            if mm_f32r:
                xTr4 = xTr_d[:].rearrange("(k p) b -> p k b", p=P)

            for t in range(T):
                b0 = t * P

                # ---- per-tile input loads (sync ring) ----
                xT_t = wpool.tile([P, 4, P], fp32, tag="xT_t")
                nc.sync.dma_start(out=xT_t[:], in_=xT4[:, :, b0 : b0 + P])
                hT_t = wpool.tile([P, 4, P], mmdt, tag="hT_t")
                nc.sync.dma_start(out=hT_t[:], in_=hT4[:, :, b0 : b0 + P])
                c_t = wpool.tile([P, H], fp32, tag="c_t")
                nc.sync.dma_start(out=c_t[:], in_=c_d[b0 : b0 + P, :])
                ptrf_t = wpool.tile([P, 1], fp32, tag="ptrf")
                nc.sync.dma_start(out=ptrf_t[:], in_=ptrf_d[b0 : b0 + P, :])
                idx_t = wpool.tile([P, 1], i32, tag="idx")
                nc.sync.dma_start(out=idx_t[:], in_=idx_d[b0 : b0 + P, :])
                wptr_t = wpool.tile([P, 1], fp32, tag="wptr")
                nc.sync.dma_start(out=wptr_t[:], in_=wptr_d[b0 : b0 + P, :])

                if t == 0:
                    # big weight loads issued after tile-0 inputs so the
                    # first tile's compute starts as early as possible
                    nc.sync.dma_start(
                        out=wg_s[:],
                        in_=wgT_d[:].rearrange("(k p) n -> p k n", p=P),
                    )
                    if has_vbias:
                        nc.sync.dma_start(out=bv_s[:], in_=biasv_d[:])

                # f32r-tagged load of x for the gates matmul (separate DMA
                # so the PE never waits on a DVE rounding copy)
                if mm_f32r:
                    xr_t = wpool.tile([P, 4, P], mmdt, tag="xr")
                    nc.sync.dma_start(out=xr_t[:], in_=xTr4[:, :, b0 : b0 + P])
                else:
                    xr_t = xT_t

                # ---- event detector:  mask = (x @ det_w.T > -det_b) ----
                det_ps = psA.tile([P, 1], fp32, tag="det")
                for k in range(4):
                    nc.tensor.matmul(
                        out=det_ps[:],
                        lhsT=xT_t[:, k, :],
                        rhs=dt_s[:, k, :],
                        start=(k == 0),
                        stop=(k == 3),
                    )
                mask_t = wpool.tile([P, 1], fp32, tag="mask")
                nc.vector.tensor_scalar(
                    out=mask_t[:], in0=det_ps[:], scalar1=neg_det_b, scalar2=None,
                    op0=Alu.is_gt,
                )

                # ---- v = x @ value_w.T (+ value_b) ----
                v_ps = psA.tile([P, D], fp32, tag="vps")
                for k in range(4):
                    nc.tensor.matmul(
                        out=v_ps[:],
                        lhsT=xT_t[:, k, :],
                        rhs=vw_s[:, k, :],
                        start=(k == 0),
                        stop=(k == 3),
                    )
                v_t = wpool.tile([P, D], fp32, tag="v")
                if has_vbias:
                    nc.vector.tensor_tensor(
                        out=v_t[:], in0=v_ps[:], in1=bv_s[:], op=Alu.add
                    )
                else:
                    nc.scalar.copy(out=v_t[:], in_=v_ps[:])

                # ---- gather slots[b, ptr[b], :] (from the *input* slots) ----
                g_t = wpool.tile([P, D], fp32, tag="g")
                nc.gpsimd.indirect_dma_start(
                    out=g_t[:],
                    out_offset=None,
                    in_=slots_d[:, :],
                    in_offset=bass.IndirectOffsetOnAxis(ap=idx_t[:, :1], axis=0),
                )

                # ---- gates: bias/x/h contributions first (no fused dep).
                # Each 512-wide PSUM bank is its own accumulation group so
                # the LSTM activations can start as soon as their bank is
                # done.  h_mem is pre-composed into the weights on the host
                # (W_comb = Wg_hmem @ proj_w), so the fused contribution is
                # the only one on the critical tail.
                gates_ps = psG.tile([P, 4 * H], fp32, tag="gates")

                def gates_mm(ft, k, kk, n, start, stop):
                    nc.tensor.matmul(
                        out=gates_ps[:, n * H : (n + 1) * H],
                        lhsT=ft[:, k, :],
                        rhs=wg_s[:, kk, n * H : (n + 1) * H],
                        start=start,
                        stop=stop,
                        skip_group_check=True,
                    )

                for n in range(4):
                    # bias row opens each bank's accumulation group
                    nc.tensor.matmul(
                        out=gates_ps[:, n * H : (n + 1) * H],
                        lhsT=ones_t[:],
                        rhs=gb_s[:, n * H : (n + 1) * H],
                        start=True,
                        stop=False,
                        skip_group_check=True,
                    )
                    for k in range(4):      # x part: kk 0..3
                        gates_mm(xr_t, k, k, n, start=False, stop=False)
                    for k in range(4):      # h part: kk 8..11
                        gates_mm(hT_t, k, 8 + k, n, start=False, stop=False)

                # ---- slots passthrough + weighted sum over s ----
                accA = wpool.tile([P, D], fp32, tag="accA")   # DVE partial
                accB = wpool.tile([P, D], fp32, tag="accB")   # GPSIMD partial
                firstA = True
                firstB = True
                store_bis = []
                for sc in range(SC):
                    sl_t = spool.tile([P, SJ, D], fp32, tag="sl")
                    nc.sync.dma_start(
                        out=sl_t[:],
                        in_=slots3[b0 : b0 + P, sc * SJ : (sc + 1) * SJ, :],
                    )
                    bi = nc.scalar.dma_start(
                        out=slots3o[b0 : b0 + P, sc * SJ : (sc + 1) * SJ, :],
                        in_=sl_t[:],
                    )
                    store_bis.append(bi)
                    for j in range(SJ):
                        s = sc * SJ + j
                        sl_j = sl_t[:, j, :]
                        if mac_on_gp[s]:
                            # uniform weights: plain add chain on GPSIMD
                            if firstB:
                                nc.gpsimd.tensor_copy(out=accB[:], in_=sl_j)
                                firstB = False
                            else:
                                nc.gpsimd.tensor_tensor(
                                    out=accB[:], in0=sl_j, in1=accB[:],
                                    op=Alu.add,
                                )
                        elif uniform_w:
                            if firstA:
                                nc.vector.tensor_copy(out=accA[:], in_=sl_j)
                                firstA = False
                            else:
                                nc.vector.tensor_tensor(
                                    out=accA[:], in0=sl_j, in1=accA[:],
                                    op=Alu.add,
                                )
                        else:
                            if firstA:
                                nc.vector.tensor_scalar(
                                    out=accA[:], in0=sl_j,
                                    scalar1=float(w_s[s]), scalar2=None,
                                    op0=Alu.mult,
                                )
                                firstA = False
                            else:
                                nc.vector.scalar_tensor_tensor(
                                    out=accA[:], in0=sl_j,
                                    scalar=float(w_s[s]), in1=accA[:],
                                    op0=Alu.mult, op1=Alu.add,
                                )

                # ---- merge partials + pointer-slot correction ----
                # corr = mask*w[ptr] * (v - gathered)
                scal_t = wpool.tile([P, 1], fp32, tag="scal")
                nc.vector.tensor_tensor(
                    out=scal_t[:], in0=mask_t[:], in1=wptr_t[:], op=Alu.mult
                )
                tmp_t = wpool.tile([P, D], fp32, tag="tmp")
                nc.vector.tensor_tensor(
                    out=tmp_t[:], in0=v_t[:], in1=g_t[:], op=Alu.subtract
                )
                nc.vector.tensor_scalar(
                    out=tmp_t[:], in0=tmp_t[:], scalar1=scal_t[:, :1],
                    scalar2=None, op0=Alu.mult,
                )
                fused_t = wpool.tile([P, D], fp32, tag="fused")
                if uniform_w:
                    # fused = w0*(accA + accB) + corr
                    nc.vector.tensor_tensor(
                        out=fused_t[:], in0=accA[:], in1=accB[:], op=Alu.add
                    )
                    nc.vector.scalar_tensor_tensor(
                        out=fused_t[:], in0=fused_t[:], scalar=float(w_s[0]),
                        in1=tmp_t[:], op0=Alu.mult, op1=Alu.add,
                    )
                else:
                    nc.vector.tensor_tensor(
                        out=fused_t[:], in0=accA[:], in1=tmp_t[:], op=Alu.add
                    )

                # ---- transpose fused -> fT (for h_mem matmul) ----
                fT_t = wpool.tile([P, 4, P], mmdt, tag="fT")
                for k in range(4):
                    tp = psA.tile([P, P], fp32, tag="tp")
                    nc.tensor.transpose(
                        out=tp[:], in_=fused_t[:, k * P : (k + 1) * P],
                        identity=ident[:],
                    )
                    nc.vector.tensor_copy(out=fT_t[:, k, :], in_=tp[:])

                # ---- gates: fused contribution (kk 4..7) closes each bank ----
                for n in range(4):
                    for k in range(4):
                        gates_mm(fT_t, k, 4 + k, n, start=False, stop=(k == 3))

                # ---- LSTM elementwise (i, f, g, o) ----
                # sigmoid(i) goes to SBUF so the i*tanh(g) multiply reads
                # only one PSUM operand; f/g/o activate in-place in PSUM.
                si_t = wpool.tile([P, H], fp32, tag="si")
                nc.scalar.activation(out=si_t[:], in_=gates_ps[:, 0:H],
                                     func=Act.Sigmoid)
                for n, fn in ((1, Act.Sigmoid), (2, Act.Tanh), (3, Act.Sigmoid)):
                    nc.scalar.activation(
                        out=gates_ps[:, n * H : (n + 1) * H],
                        in_=gates_ps[:, n * H : (n + 1) * H],
                        func=fn,
                    )
                # c_new = sig(f)*c + sig(i)*tanh(g)
                nc.vector.tensor_tensor(
                    out=c_t[:], in0=gates_ps[:, H : 2 * H], in1=c_t[:], op=Alu.mult
                )
                itg_t = wpool.tile([P, H], fp32, tag="itg")
                nc.vector.tensor_tensor(
                    out=itg_t[:], in0=gates_ps[:, 2 * H : 3 * H],
                    in1=si_t[:], op=Alu.mult,
                )
                nc.vector.tensor_tensor(
                    out=c_t[:], in0=c_t[:], in1=itg_t[:], op=Alu.add
                )
                tc_t = wpool.tile([P, H], fp32, tag="tc")
                nc.scalar.activation(out=tc_t[:], in_=c_t[:], func=Act.Tanh)
                hn_t = wpool.tile([P, H], fp32, tag="hn")
                nc.vector.tensor_tensor(
                    out=hn_t[:], in0=gates_ps[:, 3 * H : 4 * H], in1=tc_t[:],
                    op=Alu.mult,
                )
                nc.scalar.dma_start(out=c_o[b0 : b0 + P, :], in_=c_t[:])
                nc.scalar.dma_start(out=h_o[b0 : b0 + P, :], in_=hn_t[:])

                # ---- ptr_new = (ptr + mask) % S ----
                nc.vector.tensor_tensor(
                    out=ptrf_t[:], in0=ptrf_t[:], in1=mask_t[:], op=Alu.add
                )
                wrap_t = wpool.tile([P, 1], fp32, tag="wrap")
                nc.vector.tensor_scalar(
                    out=wrap_t[:], in0=ptrf_t[:], scalar1=float(S), scalar2=None,
                    op0=Alu.is_ge,
                )
                nc.vector.scalar_tensor_tensor(
                    out=ptrf_t[:], in0=wrap_t[:], scalar=-float(S), in1=ptrf_t[:],
                    op0=Alu.mult, op1=Alu.add,
                )
                pi_t = wpool.tile([P, 1], i32, tag="pi")
                nc.vector.tensor_copy(out=pi_t[:], in_=ptrf_t[:])
                nc.scalar.dma_start(out=ptr_o[b0 : b0 + P, :], in_=pi_t[:])

                # ---- scatter patch: slots_out[idx[b]] = v[b] where mask ----
                sidf_t = wpool.tile([P, 1], fp32, tag="sidf")
                nc.vector.tensor_copy(out=sidf_t[:], in_=idx_t[:])
                big_t = wpool.tile([P, 1], fp32, tag="big")
                nc.vector.tensor_scalar(
                    out=big_t[:], in0=mask_t[:], scalar1=-1.0e9, scalar2=1.0e9,
                    op0=Alu.mult, op1=Alu.add,
                )
                nc.vector.tensor_tensor(
                    out=sidf_t[:], in0=sidf_t[:], in1=big_t[:], op=Alu.add
                )
                sidi_t = wpool.tile([P, 1], i32, tag="sidi")
                nc.vector.tensor_copy(out=sidi_t[:], in_=sidf_t[:])
                sc_bi = nc.gpsimd.indirect_dma_start(
                    out=slots_o[:, :],
                    out_offset=bass.IndirectOffsetOnAxis(ap=sidi_t[:, :1], axis=0),
                    in_=v_t[:],
                    in_offset=None,
                    bounds_check=BL * S - 1,
                    oob_is_err=False,
                )
                for bi in store_bis:
                    add_dep_helper(sc_bi.ins, bi.ins,
                                   reason="scatter patch after bulk slots store")

    nc.finalize()
    return nc


def kernel(x_t, h_lstm, c_lstm, slots, ptr,
           mem_value_w, mem_value_b, mem_detector_w, mem_detector_b,
           mem_pos_emb, mem_weights, mem_proj_w, mem_proj_b,
           W_ih_w, W_ih_b, W_hh_w):
    global LAST_EXEC_NS, LAST_RESULTS
    from concourse.bass_utils import run_bass_kernel_spmd

    f32 = np.float32
    x_t = np.asarray(x_t, f32)
    h_lstm = np.asarray(h_lstm, f32)
    c_lstm = np.asarray(c_lstm, f32)
    slots = np.asarray(slots, f32)
    ptr_np = np.asarray(ptr)
    ptr_dtype = ptr_np.dtype
    ptr_np = ptr_np.astype(np.int64)

    mem_value_w = np.asarray(mem_value_w, f32)
    mem_value_b = np.asarray(mem_value_b, f32)
    mem_detector_w = np.asarray(mem_detector_w, f32)
    mem_detector_b = np.asarray(mem_detector_b, f32)
    mem_pos_emb = np.asarray(mem_pos_emb, f32)
    mem_weights = np.asarray(mem_weights, f32)
    mem_proj_w = np.asarray(mem_proj_w, f32)
    mem_proj_b = np.asarray(mem_proj_b, f32)
    W_ih_w = np.asarray(W_ih_w, f32)
    W_ih_b = np.asarray(W_ih_b, f32)
    W_hh_w = np.asarray(W_hh_w, f32)

    # ---- host-side weight prep (all tiny, O(weights)) ----
    mw = mem_weights - mem_weights.max()
    ew = np.exp(mw, dtype=f32)
    w = (ew / ew.sum(dtype=f32)).astype(f32)            # softmax(mem_weights)
    posconst = (w[:, None] * mem_pos_emb).sum(0, dtype=f32).astype(f32)  # (D,)
    proj_b_eff = (mem_proj_b + posconst @ mem_proj_w.T).astype(f32)      # (H,)

    # compose the h_mem projection into the gates weights:
    #   gates_hmem = h_mem @ Wg_m.T,  h_mem = fused @ proj_w.T + proj_b_eff
    #   => gates += fused @ (Wg_m @ proj_w).T + Wg_m @ proj_b_eff
    Wg_x = W_ih_w[:, :D]                      # (4H, D)
    Wg_m = W_ih_w[:, D : D + H]               # (4H, H)
    W_comb = (Wg_m @ mem_proj_w).astype(f32)  # (4H, D)
    gbias = (W_ih_b + Wg_m @ proj_b_eff).astype(f32)          # (4H,)
    W_eff = np.concatenate([Wg_x, W_comb, W_hh_w], axis=1)    # (4H, 1536)

    wgT = np.ascontiguousarray(W_eff.T.astype(f32))           # (1536, 2048)
    vwT = np.ascontiguousarray(mem_value_w.T)                 # (D, D)
    dtT = np.ascontiguousarray(mem_detector_w.T)              # (D, 1)
    gbrow = np.ascontiguousarray(gbias.reshape(1, 4 * H))
    has_gbias = bool(np.any(W_ih_b != 0))
    has_vbias = bool(np.any(mem_value_b != 0))

    cfg = {
        "w": tuple(float(x) for x in w),
        "neg_det_b": -float(mem_detector_b.reshape(-1)[0]),
        "has_gbias": has_gbias,
        "has_vbias": has_vbias,
        "mm_f32r": MM_F32R,
    }
    key = (tuple(cfg["w"]), cfg["neg_det_b"], has_gbias, has_vbias, MM_F32R)
    if key not in _NC_CACHE:
        _NC_CACHE[key] = _build_nc(cfg)
    nc = _NC_CACHE[key]

    # ---- shard inputs ----
    in_maps = []
    arange_sl = np.arange(BL, dtype=np.int64) * S
    for m in range(M):
        r0, r1 = m * BL, (m + 1) * BL
        pm = ptr_np[r0:r1]
        xT_m = np.ascontiguousarray(x_t[r0:r1].T)
        im = {
            "xT": xT_m,
            "hT": np.ascontiguousarray(h_lstm[r0:r1].T),
            "c": np.ascontiguousarray(c_lstm[r0:r1]),
            "slots": np.ascontiguousarray(slots[r0:r1].reshape(BL * S, D)),
            "ptrf": pm.astype(f32).reshape(BL, 1),
            "idx": (arange_sl + pm).astype(np.int32).reshape(BL, 1),
            "wptr": w[pm].astype(f32).reshape(BL, 1),
            "wgT": wgT,
            "vwT": vwT,
            "dtT": dtT,
            "gbrow": gbrow,
            "onesrow": np.ones((1, P), dtype=f32),
        }
        if MM_F32R:
            im["xTr"] = xT_m
        if has_vbias:
            im["biasv"] = np.ascontiguousarray(
                np.broadcast_to(mem_value_b[None, :], (P, D))).astype(f32)
        in_maps.append(im)

    res = run_bass_kernel_spmd(nc, in_maps, list(range(M)), trace=TRACE)
    LAST_EXEC_NS = res.exec_time_ns
    LAST_RESULTS = res

    h_new = np.concatenate([r["h_out"] for r in res.results], axis=0)
    c_new = np.concatenate([r["c_out"] for r in res.results], axis=0)
    slots_new = np.concatenate(
        [r["slots_out"].reshape(BL, S, D) for r in res.results], axis=0)
    ptr_new = np.concatenate(
        [r["ptr_out"][:, 0] for r in res.results], axis=0).astype(ptr_dtype)
    return h_new, c_new, slots_new, ptr_new


# revision 24
# speedup vs baseline: 1.0236x; 1.0236x over previous
"""EventAugmentedLSTMCell fused kernel for 8 Trainium2 NeuronCores.

Data-parallel over batch: each of the 8 cores processes 512 of the 4096
batch rows.  Host prepares transposed inputs/weights; device does all
B-proportional math:
  det  = x @ det_w.T            (PE)   -> mask = det > -det_b
  v    = x @ value_w.T (+b)     (PE)
  fused= sum_s w[s]*slots[b,s,:] + mask*w[ptr]*(v - slots[b,ptr,:])
         (+ sum_s w[s]*pos_emb[s] folded into proj bias on host)
  h_mem= fused @ proj_w.T + proj_b_eff                    (PE)
  gates= [x, h_mem, h] @ [W_ih|W_hh].T (+b)               (PE)
  LSTM elementwise                                        (ACT/DVE)
  slots_new = slots, with rows [b, ptr[b]] patched to v where mask
         (bulk passthrough SBUF copy + indirect-DMA scatter patch)
  ptr_new = (ptr + mask) % S

Scheduling notes:
 - the x/h contributions to the gates matmul are issued before the slots
   weighted-sum so only the h_mem contribution sits on the critical tail
 - slots weighted-sum MACs are split across DVE and GPSIMD (two partial
   accumulators) to shorten the per-tile reduction
 - loads go on the sync (SP) HWDGE ring, stores on the scalar (ACT) ring
   to avoid head-of-line blocking between them
"""

import numpy as np

B, D, H, S = 4096, 512, 512, 32
M = 8            # cores
BL = B // M      # 512 batch rows per core
P = 128          # partitions
T = BL // P      # 4 batch tiles per core
SJ = 4           # s-values per slots chunk
SC = S // SJ     # 8 slots chunks per batch tile

_NC_CACHE = {}
TRACE = False
MM_F32R = True
LAST_EXEC_NS = None
LAST_RESULTS = None


def _build_nc(cfg):
    import concourse.bass as bass
    import concourse.mybir as mybir
    from concourse.bacc import Bacc
    from concourse.tile import TileContext
    from concourse.masks import make_identity
    from bass_rust import add_dep_helper

    fp32 = mybir.dt.float32
    i32 = mybir.dt.int32
    Alu = mybir.AluOpType
    Act = mybir.ActivationFunctionType

    w_s = cfg["w"]                    # tuple of 32 softmax weights
    neg_det_b = cfg["neg_det_b"]
    has_gbias = cfg["has_gbias"]
    has_vbias = cfg["has_vbias"]
    mm_f32r = cfg.get("mm_f32r", False)
    uniform_w = all(x == w_s[0] for x in w_s)

    nc = Bacc()
    # float32r tensors are stored as plain f32 bytes; the PE consumes them
    # through its fast (4x) fp32 path with reduced internal precision.
    mmdt = mybir.dt.float32r if mm_f32r else fp32

    # ---- DRAM I/O ----
    xT_d = nc.dram_tensor("xT", [D, BL], fp32, kind="ExternalInput")
    if mm_f32r:
        # same bytes as xT, dtype-tagged float32r for the gates matmul
        xTr_d = nc.dram_tensor("xTr", [D, BL], mmdt, kind="ExternalInput")
    hT_d = nc.dram_tensor("hT", [H, BL], mmdt, kind="ExternalInput")
    c_d = nc.dram_tensor("c", [BL, H], fp32, kind="ExternalInput")
    slots_d = nc.dram_tensor("slots", [BL * S, D], fp32, kind="ExternalInput")
    ptrf_d = nc.dram_tensor("ptrf", [BL, 1], fp32, kind="ExternalInput")
    idx_d = nc.dram_tensor("idx", [BL, 1], i32, kind="ExternalInput")
    wptr_d = nc.dram_tensor("wptr", [BL, 1], fp32, kind="ExternalInput")
    wgT_d = nc.dram_tensor("wgT", [D + H + H, 4 * H], mmdt, kind="ExternalInput")
    vwT_d = nc.dram_tensor("vwT", [D, D], fp32, kind="ExternalInput")
    dtT_d = nc.dram_tensor("dtT", [D, 1], fp32, kind="ExternalInput")
    gbrow_d = nc.dram_tensor("gbrow", [1, 4 * H], mmdt, kind="ExternalInput")
    ones_d = nc.dram_tensor("onesrow", [1, P], mmdt, kind="ExternalInput")
    if has_vbias:
        biasv_d = nc.dram_tensor("biasv", [P, D], fp32, kind="ExternalInput")

    h_o = nc.dram_tensor("h_out", [BL, H], fp32, kind="ExternalOutput")
    c_o = nc.dram_tensor("c_out", [BL, H], fp32, kind="ExternalOutput")
    slots_o = nc.dram_tensor("slots_out", [BL * S, D], fp32, kind="ExternalOutput")
    ptr_o = nc.dram_tensor("ptr_out", [BL, 1], i32, kind="ExternalOutput")

    KG = (D + H + H) // P             # 12 contraction chunks for gates

    # engine split for the 32 weighted-sum accumulate ops (DVE ~2x GPSIMD
    # throughput).  GPSIMD can only run plain TensorTensor/copy, so it only
    # participates when the softmax weights are uniform (adds, scale folded
    # in at the merge).
    mac_on_gp = [uniform_w and (s % 3 == 2) for s in range(S)]

    with TileContext(nc) as tc:
        with (
            tc.tile_pool(name="const", bufs=1) as cpool,
            tc.tile_pool(name="work", bufs=2) as wpool,
            tc.tile_pool(name="slots", bufs=3) as spool,
            tc.tile_pool(name="psA", bufs=1, space="PSUM") as psA,
            tc.tile_pool(name="psG", bufs=1, space="PSUM") as psG,
        ):
            # ---- small constants needed by the earliest compute ----
            ident = cpool.tile([P, P], fp32)
            make_identity(nc, ident[:])

            vw_s = cpool.tile([P, 4, D], fp32)
            nc.sync.dma_start(
                out=vw_s[:], in_=vwT_d[:].rearrange("(k p) n -> p k n", p=P)
            )
            dt_s = cpool.tile([P, 4, 1], fp32)
            nc.sync.dma_start(
                out=dt_s[:], in_=dtT_d[:].rearrange("(k p) n -> p k n", p=P)
            )
            ones_t = cpool.tile([1, P], mmdt)
            nc.sync.dma_start(out=ones_t[:], in_=ones_d[:])
            gb_s = cpool.tile([1, 4 * H], mmdt)
            nc.sync.dma_start(out=gb_s[:], in_=gbrow_d[:])
            # declared now, loaded after the first tile's input DMAs
            wg_s = cpool.tile([P, KG, 4 * H], mmdt)
            if has_vbias:
                bv_s = cpool.tile([P, D], fp32)

            slots3 = slots_d[:].rearrange("(b s) d -> b s d", s=S)
            slots3o = slots_o[:].rearrange("(b s) d -> b s d", s=S)
            xT4 = xT_d[:].rearrange("(k p) b -> p k b", p=P)
            hT4 = hT_d[:].rearrange("(k p) b -> p k b", p=P)
            if mm_f32r:
                xTr4 = xTr_d[:].rearrange("(k p) b -> p k b", p=P)

            for t in range(T):
                b0 = t * P

                # ---- per-tile input loads (sync ring) ----
                xT_t = wpool.tile([P, 4, P], fp32, tag="xT_t")
                nc.sync.dma_start(out=xT_t[:], in_=xT4[:, :, b0 : b0 + P])
                hT_t = wpool.tile([P, 4, P], mmdt, tag="hT_t")
                nc.sync.dma_start(out=hT_t[:], in_=hT4[:, :, b0 : b0 + P])
                c_t = wpool.tile([P, H], fp32, tag="c_t")
                nc.sync.dma_start(out=c_t[:], in_=c_d[b0 : b0 + P, :])
                ptrf_t = wpool.tile([P, 1], fp32, tag="ptrf")
                nc.sync.dma_start(out=ptrf_t[:], in_=ptrf_d[b0 : b0 + P, :])
                idx_t = wpool.tile([P, 1], i32, tag="idx")
                nc.sync.dma_start(out=idx_t[:], in_=idx_d[b0 : b0 + P, :])
                wptr_t = wpool.tile([P, 1], fp32, tag="wptr")
                nc.sync.dma_start(out=wptr_t[:], in_=wptr_d[b0 : b0 + P, :])

                if t == 0:
                    # big weight loads issued after tile-0 inputs so the
                    # first tile's compute starts as early as possible
                    nc.sync.dma_start(
                        out=wg_s[:],
                        in_=wgT_d[:].rearrange("(k p) n -> p k n", p=P),
                    )
                    if has_vbias:
                        nc.sync.dma_start(out=bv_s[:], in_=biasv_d[:])

                # f32r-tagged load of x for the gates matmul (separate DMA
                # so the PE never waits on a DVE rounding copy)
                if mm_f32r:
                    xr_t = wpool.tile([P, 4, P], mmdt, tag="xr")
                    nc.sync.dma_start(out=xr_t[:], in_=xTr4[:, :, b0 : b0 + P])
                else:
                    xr_t = xT_t

                # ---- event detector:  mask = (x @ det_w.T > -det_b) ----
                det_ps = psA.tile([P, 1], fp32, tag="det")
                for k in range(4):
                    nc.tensor.matmul(
                        out=det_ps[:],
                        lhsT=xT_t[:, k, :],
                        rhs=dt_s[:, k, :],
                        start=(k == 0),
                        stop=(k == 3),
                    )
                mask_t = wpool.tile([P, 1], fp32, tag="mask")
                nc.vector.tensor_scalar(
                    out=mask_t[:], in0=det_ps[:], scalar1=neg_det_b, scalar2=None,
                    op0=Alu.is_gt,
                )

                # ---- v = x @ value_w.T (+ value_b) ----
                v_ps = psA.tile([P, D], fp32, tag="vps")
                for k in range(4):
                    nc.tensor.matmul(
                        out=v_ps[:],
                        lhsT=xT_t[:, k, :],
                        rhs=vw_s[:, k, :],
                        start=(k == 0),
                        stop=(k == 3),
                    )
                v_t = wpool.tile([P, D], fp32, tag="v")
                if has_vbias:
                    nc.vector.tensor_tensor(
                        out=v_t[:], in0=v_ps[:], in1=bv_s[:], op=Alu.add
                    )
                else:
                    nc.scalar.copy(out=v_t[:], in_=v_ps[:])

                # ---- gather slots[b, ptr[b], :] (from the *input* slots) ----
                g_t = wpool.tile([P, D], fp32, tag="g")
                nc.gpsimd.indirect_dma_start(
                    out=g_t[:],
                    out_offset=None,
                    in_=slots_d[:, :],
                    in_offset=bass.IndirectOffsetOnAxis(ap=idx_t[:, :1], axis=0),
                )

                # ---- gates: bias/x/h contributions first (no fused dep).
                # Each 512-wide PSUM bank is its own accumulation group so
                # the LSTM activations can start as soon as their bank is
                # done.  h_mem is pre-composed into the weights on the host
                # (W_comb = Wg_hmem @ proj_w), so the fused contribution is
                # the only one on the critical tail.
                gates_ps = psG.tile([P, 4 * H], fp32, tag="gates")

                def gates_mm(ft, k, kk, n, start, stop):
                    nc.tensor.matmul(
                        out=gates_ps[:, n * H : (n + 1) * H],
                        lhsT=ft[:, k, :],
                        rhs=wg_s[:, kk, n * H : (n + 1) * H],
                        start=start,
                        stop=stop,
                        skip_group_check=True,
                    )

                for n in range(4):
                    # bias row opens each bank's accumulation group
                    nc.tensor.matmul(
                        out=gates_ps[:, n * H : (n + 1) * H],
                        lhsT=ones_t[:],
                        rhs=gb_s[:, n * H : (n + 1) * H],
                        start=True,
                        stop=False,
                        skip_group_check=True,
                    )
                    for k in range(4):      # x part: kk 0..3
                        gates_mm(xr_t, k, k, n, start=False, stop=False)
                    for k in range(4):      # h part: kk 8..11
                        gates_mm(hT_t, k, 8 + k, n, start=False, stop=False)

                # ---- slots passthrough + weighted sum over s ----
                accA = wpool.tile([P, D], fp32, tag="accA")   # DVE partial
                accB = wpool.tile([P, D], fp32, tag="accB")   # GPSIMD partial
                firstA = True
                firstB = True
                store_bis = []
                for sc in range(SC):
                    sl_t = spool.tile([P, SJ, D], fp32, tag="sl")
                    nc.sync.dma_start(
                        out=sl_t[:],
                        in_=slots3[b0 : b0 + P, sc * SJ : (sc + 1) * SJ, :],
                    )
                    bi = nc.scalar.dma_start(
                        out=slots3o[b0 : b0 + P, sc * SJ : (sc + 1) * SJ, :],
                        in_=sl_t[:],
                    )
                    store_bis.append(bi)
                    for j in range(SJ):
                        s = sc * SJ + j
                        sl_j = sl_t[:, j, :]
                        if mac_on_gp[s]:
                            # uniform weights: plain add chain on GPSIMD
                            if firstB:
                                nc.gpsimd.tensor_copy(out=accB[:], in_=sl_j)
                                firstB = False
                            else:
                                nc.gpsimd.tensor_tensor(
                                    out=accB[:], in0=sl_j, in1=accB[:],
                                    op=Alu.add,
                                )
                        elif uniform_w:
                            if firstA:
                                nc.vector.tensor_copy(out=accA[:], in_=sl_j)
                                firstA = False
                            else:
                                nc.vector.tensor_tensor(
                                    out=accA[:], in0=sl_j, in1=accA[:],
                                    op=Alu.add,
                                )
                        else:
                            if firstA:
                                nc.vector.tensor_scalar(
                                    out=accA[:], in0=sl_j,
                                    scalar1=float(w_s[s]), scalar2=None,
                                    op0=Alu.mult,
                                )
                                firstA = False
                            else:
                                nc.vector.scalar_tensor_tensor(
                                    out=accA[:], in0=sl_j,
                                    scalar=float(w_s[s]), in1=accA[:],
                                    op0=Alu.mult, op1=Alu.add,
                                )

                # ---- merge partials + pointer-slot correction ----
                # corr = mask*w[ptr] * (v - gathered)
                scal_t = wpool.tile([P, 1], fp32, tag="scal")
                nc.vector.tensor_tensor(
                    out=scal_t[:], in0=mask_t[:], in1=wptr_t[:], op=Alu.mult
                )
                tmp_t = wpool.tile([P, D], fp32, tag="tmp")
                nc.vector.tensor_tensor(
                    out=tmp_t[:], in0=v_t[:], in1=g_t[:], op=Alu.subtract
                )
                nc.vector.tensor_scalar(
                    out=tmp_t[:], in0=tmp_t[:], scalar1=scal_t[:, :1],
                    scalar2=None, op0=Alu.mult,
                )
                fused_t = wpool.tile([P, D], fp32, tag="fused")
                if uniform_w:
                    # fused = w0*(accA + accB) + corr
                    nc.vector.tensor_tensor(
                        out=fused_t[:], in0=accA[:], in1=accB[:], op=Alu.add
                    )
                    nc.vector.scalar_tensor_tensor(
                        out=fused_t[:], in0=fused_t[:], scalar=float(w_s[0]),
                        in1=tmp_t[:], op0=Alu.mult, op1=Alu.add,
                    )
                else:
                    nc.vector.tensor_tensor(
                        out=fused_t[:], in0=accA[:], in1=tmp_t[:], op=Alu.add
                    )

                # ---- transpose fused -> fT (for h_mem matmul) ----
                fT_t = wpool.tile([P, 4, P], mmdt, tag="fT")
                for k in range(4):
                    tp = psA.tile([P, P], fp32, tag="tp")
                    nc.tensor.transpose(
                        out=tp[:], in_=fused_t[:, k * P : (k + 1) * P],
                        identity=ident[:],
                    )
                    nc.vector.tensor_copy(out=fT_t[:, k, :], in_=tp[:])

                # ---- gates: fused contribution (kk 4..7) closes each bank ----
                for n in range(4):
                    for k in range(4):
                        gates_mm(fT_t, k, 4 + k, n, start=False, stop=(k == 3))

                # ---- LSTM elementwise (i, f, g, o) ----
                # sigmoid(i) goes to SBUF so the i*tanh(g) multiply reads
                # only one PSUM operand; f/g/o activate in-place in PSUM.
                si_t = wpool.tile([P, H], fp32, tag="si")
                nc.scalar.activation(out=si_t[:], in_=gates_ps[:, 0:H],
                                     func=Act.Sigmoid)
                for n, fn in ((1, Act.Sigmoid), (2, Act.Tanh), (3, Act.Sigmoid)):
                    nc.scalar.activation(
                        out=gates_ps[:, n * H : (n + 1) * H],
                        in_=gates_ps[:, n * H : (n + 1) * H],
                        func=fn,
                    )
                # c_new = sig(f)*c + sig(i)*tanh(g)
                nc.vector.tensor_tensor(
                    out=c_t[:], in0=gates_ps[:, H : 2 * H], in1=c_t[:], op=Alu.mult
                )
                itg_t = wpool.tile([P, H], fp32, tag="itg")
                nc.vector.tensor_tensor(
                    out=itg_t[:], in0=gates_ps[:, 2 * H : 3 * H],
                    in1=si_t[:], op=Alu.mult,
                )
                nc.vector.tensor_tensor(
                    out=c_t[:], in0=c_t[:], in1=itg_t[:], op=Alu.add
                )
                tc_t = wpool.tile([P, H], fp32, tag="tc")
                nc.scalar.activation(out=tc_t[:], in_=c_t[:], func=Act.Tanh)
                hn_t = wpool.tile([P, H], fp32, tag="hn")
                nc.vector.tensor_tensor(
                    out=hn_t[:], in0=gates_ps[:, 3 * H : 4 * H], in1=tc_t[:],
                    op=Alu.mult,
                )
                nc.scalar.dma_start(out=c_o[b0 : b0 + P, :], in_=c_t[:])
                nc.scalar.dma_start(out=h_o[b0 : b0 + P, :], in_=hn_t[:])

                # ---- ptr_new = (ptr + mask) % S ----
                nc.vector.tensor_tensor(
                    out=ptrf_t[:], in0=ptrf_t[:], in1=mask_t[:], op=Alu.add
                )
                wrap_t = wpool.tile([P, 1], fp32, tag="wrap")
                nc.vector.tensor_scalar(
                    out=wrap_t[:], in0=ptrf_t[:], scalar1=float(S), scalar2=None,
                    op0=Alu.is_ge,
                )
                nc.vector.scalar_tensor_tensor(
                    out=ptrf_t[:], in0=wrap_t[:], scalar=-float(S), in1=ptrf_t[:],
                    op0=Alu.mult, op1=Alu.add,
                )
                pi_t = wpool.tile([P, 1], i32, tag="pi")
                nc.vector.tensor_copy(out=pi_t[:], in_=ptrf_t[:])
                nc.scalar.dma_start(out=ptr_o[b0 : b0 + P, :], in_=pi_t[:])

                # ---- scatter patch: slots_out[idx[b]] = v[b] where mask ----
                sidf_t = wpool.tile([P, 1], fp32, tag="sidf")
                nc.vector.tensor_copy(out=sidf_t[:], in_=idx_t[:])
                big_t = wpool.tile([P, 1], fp32, tag="big")
                nc.vector.tensor_scalar(
                    out=big_t[:], in0=mask_t[:], scalar1=-1.0e9, scalar2=1.0e9,
                    op0=Alu.mult, op1=Alu.add,
                )
                nc.vector.tensor_tensor(
                    out=sidf_t[:], in0=sidf_t[:], in1=big_t[:], op=Alu.add
                )
                sidi_t = wpool.tile([P, 1], i32, tag="sidi")
                nc.vector.tensor_copy(out=sidi_t[:], in_=sidf_t[:])
                sc_bi = nc.gpsimd.indirect_dma_start(
                    out=slots_o[:, :],
                    out_offset=bass.IndirectOffsetOnAxis(ap=sidi_t[:, :1], axis=0),
                    in_=v_t[:],
                    in_offset=None,
                    bounds_check=BL * S - 1,
                    oob_is_err=False,
                )
                for bi in store_bis:
                    add_dep_helper(sc_bi.ins, bi.ins,
                                   reason="scatter patch after bulk slots store")

    nc.finalize()
    return nc


def kernel(x_t, h_lstm, c_lstm, slots, ptr,
           mem_value_w, mem_value_b, mem_detector_w, mem_detector_b,
           mem_pos_emb, mem_weights, mem_proj_w, mem_proj_b,
           W_ih_w, W_ih_b, W_hh_w):
    global LAST_EXEC_NS, LAST_RESULTS
    from concourse.bass_utils import run_bass_kernel_spmd

    f32 = np.float32
    x_t = np.asarray(x_t, f32)
    h_lstm = np.asarray(h_lstm, f32)
    c_lstm = np.asarray(c_lstm, f32)
    slots = np.asarray(slots, f32)
    ptr_np = np.asarray(ptr)
    ptr_dtype = ptr_np.dtype
    ptr_np = ptr_np.astype(np.int64)

    mem_value_w = np.asarray(mem_value_w, f32)
    mem_value_b = np.asarray(mem_value_b, f32)
    mem_detector_w = np.asarray(mem_detector_w, f32)
    mem_detector_b = np.asarray(mem_detector_b, f32)
    mem_pos_emb = np.asarray(mem_pos_emb, f32)
    mem_weights = np.asarray(mem_weights, f32)
    mem_proj_w = np.asarray(mem_proj_w, f32)
    mem_proj_b = np.asarray(mem_proj_b, f32)
    W_ih_w = np.asarray(W_ih_w, f32)
    W_ih_b = np.asarray(W_ih_b, f32)
    W_hh_w = np.asarray(W_hh_w, f32)

    # ---- host-side weight prep (all tiny, O(weights)) ----
    mw = mem_weights - mem_weights.max()
    ew = np.exp(mw, dtype=f32)
    w = (ew / ew.sum(dtype=f32)).astype(f32)            # softmax(mem_weights)
    posconst = (w[:, None] * mem_pos_emb).sum(0, dtype=f32).astype(f32)  # (D,)
    proj_b_eff = (mem_proj_b + posconst @ mem_proj_w.T).astype(f32)      # (H,)

    # compose the h_mem projection into the gates weights:
    #   gates_hmem = h_mem @ Wg_m.T,  h_mem = fused @ proj_w.T + proj_b_eff
    #   => gates += fused @ (Wg_m @ proj_w).T + Wg_m @ proj_b_eff
    Wg_x = W_ih_w[:, :D]                      # (4H, D)
    Wg_m = W_ih_w[:, D : D + H]               # (4H, H)
    W_comb = (Wg_m @ mem_proj_w).astype(f32)  # (4H, D)
    gbias = (W_ih_b + Wg_m @ proj_b_eff).astype(f32)          # (4H,)
    W_eff = np.concatenate([Wg_x, W_comb, W_hh_w], axis=1)    # (4H, 1536)

    wgT = np.ascontiguousarray(W_eff.T.astype(f32))           # (1536, 2048)
    vwT = np.ascontiguousarray(mem_value_w.T)                 # (D, D)
    dtT = np.ascontiguousarray(mem_detector_w.T)              # (D, 1)
    gbrow = np.ascontiguousarray(gbias.reshape(1, 4 * H))
    has_gbias = bool(np.any(W_ih_b != 0))
    has_vbias = bool(np.any(mem_value_b != 0))

    cfg = {
        "w": tuple(float(x) for x in w),
        "neg_det_b": -float(mem_detector_b.reshape(-1)[0]),
        "has_gbias": has_gbias,
        "has_vbias": has_vbias,
        "mm_f32r": MM_F32R,
    }
    key = (tuple(cfg["w"]), cfg["neg_det_b"], has_gbias, has_vbias, MM_F32R)
    if key not in _NC_CACHE:
        _NC_CACHE[key] = _build_nc(cfg)
    nc = _NC_CACHE[key]

    # ---- shard inputs ----
    in_maps = []
    arange_sl = np.arange(BL, dtype=np.int64) * S
    for m in range(M):
        r0, r1 = m * BL, (m + 1) * BL
        pm = ptr_np[r0:r1]
        xT_m = np.ascontiguousarray(x_t[r0:r1].T)
        im = {
            "xT": xT_m,
            "hT": np.ascontiguousarray(h_lstm[r0:r1].T),
            "c": np.ascontiguousarray(c_lstm[r0:r1]),
            "slots": np.ascontiguousarray(slots[r0:r1].reshape(BL * S, D)),
            "ptrf": pm.astype(f32).reshape(BL, 1),
            "idx": (arange_sl + pm).astype(np.int32).reshape(BL, 1),
            "wptr": w[pm].astype(f32).reshape(BL, 1),
            "wgT": wgT,
            "vwT": vwT,
            "dtT": dtT,
            "gbrow": gbrow,
            "onesrow": np.ones((1, P), dtype=f32),
        }
        if MM_F32R:
            im["xTr"] = xT_m
        if has_vbias:
            im["biasv"] = np.ascontiguousarray(
                np.broadcast_to(mem_value_b[None, :], (P, D))).astype(f32)
        in_maps.append(im)

    res = run_bass_kernel_spmd(nc, in_maps, list(range(M)), trace=TRACE)
    LAST_EXEC_NS = res.exec_time_ns
    LAST_RESULTS = res

    h_new = np.concatenate([r["h_out"] for r in res.results], axis=0)
    c_new = np.concatenate([r["c_out"] for r in res.results], axis=0)
    slots_new = np.concatenate(
        [r["slots_out"].reshape(BL, S, D) for r in res.results], axis=0)
    ptr_new = np.concatenate(
        [r["ptr_out"][:, 0] for r in res.results], axis=0).astype(ptr_dtype)
    return h_new, c_new, slots_new, ptr_new


# revision 25
# speedup vs baseline: 1.1429x; 1.1165x over previous
"""EventAugmentedLSTMCell fused kernel for 8 Trainium2 NeuronCores.

Data-parallel over batch: each of the 8 cores processes 512 of the 4096
batch rows.  Host prepares transposed inputs/weights; device does all
B-proportional math:
  det  = x @ det_w.T            (PE)   -> mask = det > -det_b
  v    = x @ value_w.T (+b)     (PE)
  fused= sum_s w[s]*slots[b,s,:] + mask*w[ptr]*(v - slots[b,ptr,:])
         (+ sum_s w[s]*pos_emb[s] folded into proj bias on host)
  h_mem= fused @ proj_w.T + proj_b_eff                    (PE)
  gates= [x, h_mem, h] @ [W_ih|W_hh].T (+b)               (PE)
  LSTM elementwise                                        (ACT/DVE)
  slots_new = slots, with rows [b, ptr[b]] patched to v where mask
         (bulk passthrough SBUF copy + indirect-DMA scatter patch)
  ptr_new = (ptr + mask) % S

Scheduling notes:
 - the x/h contributions to the gates matmul are issued before the slots
   weighted-sum so only the h_mem contribution sits on the critical tail
 - slots weighted-sum MACs are split across DVE and GPSIMD (two partial
   accumulators) to shorten the per-tile reduction
 - loads go on the sync (SP) HWDGE ring, stores on the scalar (ACT) ring
   to avoid head-of-line blocking between them
"""

import numpy as np

B, D, H, S = 4096, 512, 512, 32
M = 8            # cores
BL = B // M      # 512 batch rows per core
P = 128          # partitions
T = BL // P      # 4 batch tiles per core
SJ = 4           # s-values per slots chunk
SC = S // SJ     # 8 slots chunks per batch tile

_NC_CACHE = {}
TRACE = False
MM_F32R = True
LAST_EXEC_NS = None
LAST_RESULTS = None


def _build_nc(cfg):
    import concourse.bass as bass
    import concourse.mybir as mybir
    from concourse.bacc import Bacc
    from concourse.tile import TileContext
    from concourse.masks import make_identity
    from bass_rust import add_dep_helper

    fp32 = mybir.dt.float32
    i32 = mybir.dt.int32
    Alu = mybir.AluOpType
    Act = mybir.ActivationFunctionType

    w_s = cfg["w"]                    # tuple of 32 softmax weights
    neg_det_b = cfg["neg_det_b"]
    has_gbias = cfg["has_gbias"]
    has_vbias = cfg["has_vbias"]
    mm_f32r = cfg.get("mm_f32r", False)
    uniform_w = all(x == w_s[0] for x in w_s)

    nc = Bacc()
    # float32r tensors are stored as plain f32 bytes; the PE consumes them
    # through its fast (4x) fp32 path with reduced internal precision.
    mmdt = mybir.dt.float32r if mm_f32r else fp32

    # ---- DRAM I/O ----
    xT_d = nc.dram_tensor("xT", [D, BL], fp32, kind="ExternalInput")
    hT_d = nc.dram_tensor("hT", [H, BL], mmdt, kind="ExternalInput")
    c_d = nc.dram_tensor("c", [BL, H], fp32, kind="ExternalInput")
    slots_d = nc.dram_tensor("slots", [BL * S, D], fp32, kind="ExternalInput")
    ptrf_d = nc.dram_tensor("ptrf", [BL, 1], fp32, kind="ExternalInput")
    idx_d = nc.dram_tensor("idx", [BL, 1], i32, kind="ExternalInput")
    wptr_d = nc.dram_tensor("wptr", [BL, 1], fp32, kind="ExternalInput")
    wgT_d = nc.dram_tensor("wgT", [D + H + H, 4 * H], mmdt, kind="ExternalInput")
    vwT_d = nc.dram_tensor("vwT", [D, D], fp32, kind="ExternalInput")
    dtT_d = nc.dram_tensor("dtT", [D, 1], fp32, kind="ExternalInput")
    gbrow_d = nc.dram_tensor("gbrow", [1, 4 * H], mmdt, kind="ExternalInput")
    ones_d = nc.dram_tensor("onesrow", [1, P], mmdt, kind="ExternalInput")
    if has_vbias:
        biasv_d = nc.dram_tensor("biasv", [P, D], fp32, kind="ExternalInput")

    h_o = nc.dram_tensor("h_out", [BL, H], fp32, kind="ExternalOutput")
    c_o = nc.dram_tensor("c_out", [BL, H], fp32, kind="ExternalOutput")
    slots_o = nc.dram_tensor("slots_out", [BL * S, D], fp32, kind="ExternalOutput")
    ptr_o = nc.dram_tensor("ptr_out", [BL, 1], i32, kind="ExternalOutput")

    KG = (D + H + H) // P             # 12 contraction chunks for gates

    # engine split for the 32 weighted-sum accumulate ops (DVE ~2x GPSIMD
    # throughput).  GPSIMD can only run plain TensorTensor/copy, so it only
    # participates when the softmax weights are uniform (adds, scale folded
    # in at the merge).
    mac_on_gp = [uniform_w and (s % 3 == 2) for s in range(S)]

    with TileContext(nc) as tc:
        with (
            tc.tile_pool(name="const", bufs=1) as cpool,
            tc.tile_pool(name="work", bufs=2) as wpool,
            tc.tile_pool(name="slots", bufs=3) as spool,
            tc.tile_pool(name="psA", bufs=1, space="PSUM") as psA,
            tc.tile_pool(name="psG", bufs=1, space="PSUM") as psG,
        ):
            # ---- small constants needed by the earliest compute ----
            ident = cpool.tile([P, P], fp32)
            make_identity(nc, ident[:])

            vw_s = cpool.tile([P, 4, D], fp32)
            nc.sync.dma_start(
                out=vw_s[:], in_=vwT_d[:].rearrange("(k p) n -> p k n", p=P)
            )
            dt_s = cpool.tile([P, 4, 1], fp32)
            nc.sync.dma_start(
                out=dt_s[:], in_=dtT_d[:].rearrange("(k p) n -> p k n", p=P)
            )
            ones_t = cpool.tile([1, P], mmdt)
            nc.sync.dma_start(out=ones_t[:], in_=ones_d[:])
            gb_s = cpool.tile([1, 4 * H], mmdt)
            nc.sync.dma_start(out=gb_s[:], in_=gbrow_d[:])
            # declared now, loaded after the first tile's input DMAs
            wg_s = cpool.tile([P, KG, 4 * H], mmdt)
            if has_vbias:
                bv_s = cpool.tile([P, D], fp32)

            slots3 = slots_d[:].rearrange("(b s) d -> b s d", s=S)
            slots3o = slots_o[:].rearrange("(b s) d -> b s d", s=S)
            xT4 = xT_d[:].rearrange("(k p) b -> p k b", p=P)
            hT4 = hT_d[:].rearrange("(k p) b -> p k b", p=P)

            for t in range(T):
                b0 = t * P

                # ---- per-tile input loads (sync ring) ----
                xT_t = wpool.tile([P, 4, P], fp32, tag="xT_t")
                nc.sync.dma_start(out=xT_t[:], in_=xT4[:, :, b0 : b0 + P])
                hT_t = wpool.tile([P, 4, P], mmdt, tag="hT_t")
                nc.sync.dma_start(out=hT_t[:], in_=hT4[:, :, b0 : b0 + P])
                c_t = wpool.tile([P, H], fp32, tag="c_t")
                nc.sync.dma_start(out=c_t[:], in_=c_d[b0 : b0 + P, :])
                ptrf_t = wpool.tile([P, 1], fp32, tag="ptrf")
                nc.sync.dma_start(out=ptrf_t[:], in_=ptrf_d[b0 : b0 + P, :])
                idx_t = wpool.tile([P, 1], i32, tag="idx")
                nc.sync.dma_start(out=idx_t[:], in_=idx_d[b0 : b0 + P, :])
                wptr_t = wpool.tile([P, 1], fp32, tag="wptr")
                nc.sync.dma_start(out=wptr_t[:], in_=wptr_d[b0 : b0 + P, :])

                if t == 0:
                    # big weight loads issued after tile-0 inputs so the
                    # first tile's compute starts as early as possible
                    nc.sync.dma_start(
                        out=wg_s[:],
                        in_=wgT_d[:].rearrange("(k p) n -> p k n", p=P),
                    )
                    if has_vbias:
                        nc.sync.dma_start(out=bv_s[:], in_=biasv_d[:])

                # f32r rounding copy of x for the gates matmul, on the
                # lightly-loaded ACT engine so it is scheduled early and the
                # PE never queues behind the DVE slots reduction
                if mm_f32r:
                    xr_t = wpool.tile([P, 4, P], mmdt, tag="xr")
                    nc.scalar.copy(out=xr_t[:], in_=xT_t[:])
                else:
                    xr_t = xT_t

                # ---- event detector:  mask = (x @ det_w.T > -det_b) ----
                det_ps = psA.tile([P, 1], fp32, tag="det")
                for k in range(4):
                    nc.tensor.matmul(
                        out=det_ps[:],
                        lhsT=xT_t[:, k, :],
                        rhs=dt_s[:, k, :],
                        start=(k == 0),
                        stop=(k == 3),
                    )
                mask_t = wpool.tile([P, 1], fp32, tag="mask")
                nc.vector.tensor_scalar(
                    out=mask_t[:], in0=det_ps[:], scalar1=neg_det_b, scalar2=None,
                    op0=Alu.is_gt,
                )

                # ---- v = x @ value_w.T (+ value_b) ----
                v_ps = psA.tile([P, D], fp32, tag="vps")
                for k in range(4):
                    nc.tensor.matmul(
                        out=v_ps[:],
                        lhsT=xT_t[:, k, :],
                        rhs=vw_s[:, k, :],
                        start=(k == 0),
                        stop=(k == 3),
                    )
                v_t = wpool.tile([P, D], fp32, tag="v")
                if has_vbias:
                    nc.vector.tensor_tensor(
                        out=v_t[:], in0=v_ps[:], in1=bv_s[:], op=Alu.add
                    )
                else:
                    nc.scalar.copy(out=v_t[:], in_=v_ps[:])

                # ---- gather slots[b, ptr[b], :] (from the *input* slots) ----
                g_t = wpool.tile([P, D], fp32, tag="g")
                nc.gpsimd.indirect_dma_start(
                    out=g_t[:],
                    out_offset=None,
                    in_=slots_d[:, :],
                    in_offset=bass.IndirectOffsetOnAxis(ap=idx_t[:, :1], axis=0),
                )

                # ---- gates: bias/x/h contributions first (no fused dep).
                # Each 512-wide PSUM bank is its own accumulation group so
                # the LSTM activations can start as soon as their bank is
                # done.  h_mem is pre-composed into the weights on the host
                # (W_comb = Wg_hmem @ proj_w), so the fused contribution is
                # the only one on the critical tail.
                gates_ps = psG.tile([P, 4 * H], fp32, tag="gates")

                def gates_mm(ft, k, kk, n, start, stop):
                    nc.tensor.matmul(
                        out=gates_ps[:, n * H : (n + 1) * H],
                        lhsT=ft[:, k, :],
                        rhs=wg_s[:, kk, n * H : (n + 1) * H],
                        start=start,
                        stop=stop,
                        skip_group_check=True,
                    )

                for n in range(4):
                    # bias row opens each bank's accumulation group
                    nc.tensor.matmul(
                        out=gates_ps[:, n * H : (n + 1) * H],
                        lhsT=ones_t[:],
                        rhs=gb_s[:, n * H : (n + 1) * H],
                        start=True,
                        stop=False,
                        skip_group_check=True,
                    )
                    for k in range(4):      # x part: kk 0..3
                        gates_mm(xr_t, k, k, n, start=False, stop=False)
                    for k in range(4):      # h part: kk 8..11
                        gates_mm(hT_t, k, 8 + k, n, start=False, stop=False)

                # ---- slots passthrough + weighted sum over s ----
                accA = wpool.tile([P, D], fp32, tag="accA")   # DVE partial
                accB = wpool.tile([P, D], fp32, tag="accB")   # GPSIMD partial
                firstA = True
                firstB = True
                store_bis = []
                for sc in range(SC):
                    sl_t = spool.tile([P, SJ, D], fp32, tag="sl")
                    nc.sync.dma_start(
                        out=sl_t[:],
                        in_=slots3[b0 : b0 + P, sc * SJ : (sc + 1) * SJ, :],
                    )
                    bi = nc.scalar.dma_start(
                        out=slots3o[b0 : b0 + P, sc * SJ : (sc + 1) * SJ, :],
                        in_=sl_t[:],
                    )
                    store_bis.append(bi)
                    for j in range(SJ):
                        s = sc * SJ + j
                        sl_j = sl_t[:, j, :]
                        if mac_on_gp[s]:
                            # uniform weights: plain add chain on GPSIMD
                            if firstB:
                                nc.gpsimd.tensor_copy(out=accB[:], in_=sl_j)
                                firstB = False
                            else:
                                nc.gpsimd.tensor_tensor(
                                    out=accB[:], in0=sl_j, in1=accB[:],
                                    op=Alu.add,
                                )
                        elif uniform_w:
                            if firstA:
                                nc.vector.tensor_copy(out=accA[:], in_=sl_j)
                                firstA = False
                            else:
                                nc.vector.tensor_tensor(
                                    out=accA[:], in0=sl_j, in1=accA[:],
                                    op=Alu.add,
                                )
                        else:
                            if firstA:
                                nc.vector.tensor_scalar(
                                    out=accA[:], in0=sl_j,
                                    scalar1=float(w_s[s]), scalar2=None,
                                    op0=Alu.mult,
                                )
                                firstA = False
                            else:
                                nc.vector.scalar_tensor_tensor(
                                    out=accA[:], in0=sl_j,
                                    scalar=float(w_s[s]), in1=accA[:],
                                    op0=Alu.mult, op1=Alu.add,
                                )

                # ---- merge partials + pointer-slot correction ----
                # corr = mask*w[ptr] * (v - gathered)
                scal_t = wpool.tile([P, 1], fp32, tag="scal")
                nc.vector.tensor_tensor(
                    out=scal_t[:], in0=mask_t[:], in1=wptr_t[:], op=Alu.mult
                )
                tmp_t = wpool.tile([P, D], fp32, tag="tmp")
                nc.vector.tensor_tensor(
                    out=tmp_t[:], in0=v_t[:], in1=g_t[:], op=Alu.subtract
                )
                nc.vector.tensor_scalar(
                    out=tmp_t[:], in0=tmp_t[:], scalar1=scal_t[:, :1],
                    scalar2=None, op0=Alu.mult,
                )
                fused_t = wpool.tile([P, D], fp32, tag="fused")
                if uniform_w:
                    # fused = w0*(accA + accB) + corr
                    nc.vector.tensor_tensor(
                        out=fused_t[:], in0=accA[:], in1=accB[:], op=Alu.add
                    )
                    nc.vector.scalar_tensor_tensor(
                        out=fused_t[:], in0=fused_t[:], scalar=float(w_s[0]),
                        in1=tmp_t[:], op0=Alu.mult, op1=Alu.add,
                    )
                else:
                    nc.vector.tensor_tensor(
                        out=fused_t[:], in0=accA[:], in1=tmp_t[:], op=Alu.add
                    )

                # ---- transpose fused -> fT (for h_mem matmul) ----
                fT_t = wpool.tile([P, 4, P], mmdt, tag="fT")
                for k in range(4):
                    tp = psA.tile([P, P], fp32, tag="tp")
                    nc.tensor.transpose(
                        out=tp[:], in_=fused_t[:, k * P : (k + 1) * P],
                        identity=ident[:],
                    )
                    nc.vector.tensor_copy(out=fT_t[:, k, :], in_=tp[:])

                # ---- gates: fused contribution (kk 4..7) closes each bank ----
                for n in range(4):
                    for k in range(4):
                        gates_mm(fT_t, k, 4 + k, n, start=False, stop=(k == 3))

                # ---- LSTM elementwise (i, f, g, o) ----
                # sigmoid(i) goes to SBUF so the i*tanh(g) multiply reads
                # only one PSUM operand; f/g/o activate in-place in PSUM.
                si_t = wpool.tile([P, H], fp32, tag="si")
                nc.scalar.activation(out=si_t[:], in_=gates_ps[:, 0:H],
                                     func=Act.Sigmoid)
                for n, fn in ((1, Act.Sigmoid), (2, Act.Tanh), (3, Act.Sigmoid)):
                    nc.scalar.activation(
                        out=gates_ps[:, n * H : (n + 1) * H],
                        in_=gates_ps[:, n * H : (n + 1) * H],
                        func=fn,
                    )
                # c_new = sig(f)*c + sig(i)*tanh(g)
                nc.vector.tensor_tensor(
                    out=c_t[:], in0=gates_ps[:, H : 2 * H], in1=c_t[:], op=Alu.mult
                )
                itg_t = wpool.tile([P, H], fp32, tag="itg")
                nc.vector.tensor_tensor(
                    out=itg_t[:], in0=gates_ps[:, 2 * H : 3 * H],
                    in1=si_t[:], op=Alu.mult,
                )
                nc.vector.tensor_tensor(
                    out=c_t[:], in0=c_t[:], in1=itg_t[:], op=Alu.add
                )
                tc_t = wpool.tile([P, H], fp32, tag="tc")
                nc.scalar.activation(out=tc_t[:], in_=c_t[:], func=Act.Tanh)
                hn_t = wpool.tile([P, H], fp32, tag="hn")
                nc.vector.tensor_tensor(
                    out=hn_t[:], in0=gates_ps[:, 3 * H : 4 * H], in1=tc_t[:],
                    op=Alu.mult,
                )
                nc.scalar.dma_start(out=c_o[b0 : b0 + P, :], in_=c_t[:])
                nc.scalar.dma_start(out=h_o[b0 : b0 + P, :], in_=hn_t[:])

                # ---- ptr_new = (ptr + mask) % S ----
                nc.vector.tensor_tensor(
                    out=ptrf_t[:], in0=ptrf_t[:], in1=mask_t[:], op=Alu.add
                )
                wrap_t = wpool.tile([P, 1], fp32, tag="wrap")
                nc.vector.tensor_scalar(
                    out=wrap_t[:], in0=ptrf_t[:], scalar1=float(S), scalar2=None,
                    op0=Alu.is_ge,
                )
                nc.vector.scalar_tensor_tensor(
                    out=ptrf_t[:], in0=wrap_t[:], scalar=-float(S), in1=ptrf_t[:],
                    op0=Alu.mult, op1=Alu.add,
                )
                pi_t = wpool.tile([P, 1], i32, tag="pi")
                nc.vector.tensor_copy(out=pi_t[:], in_=ptrf_t[:])
                nc.scalar.dma_start(out=ptr_o[b0 : b0 + P, :], in_=pi_t[:])

                # ---- scatter patch: slots_out[idx[b]] = v[b] where mask ----
                sidf_t = wpool.tile([P, 1], fp32, tag="sidf")
                nc.vector.tensor_copy(out=sidf_t[:], in_=idx_t[:])
                big_t = wpool.tile([P, 1], fp32, tag="big")
                nc.vector.tensor_scalar(
                    out=big_t[:], in0=mask_t[:], scalar1=-1.0e9, scalar2=1.0e9,
                    op0=Alu.mult, op1=Alu.add,
                )
                nc.vector.tensor_tensor(
                    out=sidf_t[:], in0=sidf_t[:], in1=big_t[:], op=Alu.add
                )
                sidi_t = wpool.tile([P, 1], i32, tag="sidi")
                nc.vector.tensor_copy(out=sidi_t[:], in_=sidf_t[:])
                sc_bi = nc.gpsimd.indirect_dma_start(
                    out=slots_o[:, :],
                    out_offset=bass.IndirectOffsetOnAxis(ap=sidi_t[:, :1], axis=0),
                    in_=v_t[:],
                    in_offset=None,
                    bounds_check=BL * S - 1,
                    oob_is_err=False,
                )
                for bi in store_bis:
                    add_dep_helper(sc_bi.ins, bi.ins,
                                   reason="scatter patch after bulk slots store")

    nc.finalize()
    return nc


def kernel(x_t, h_lstm, c_lstm, slots, ptr,
           mem_value_w, mem_value_b, mem_detector_w, mem_detector_b,
           mem_pos_emb, mem_weights, mem_proj_w, mem_proj_b,
           W_ih_w, W_ih_b, W_hh_w):
    global LAST_EXEC_NS, LAST_RESULTS
    from concourse.bass_utils import run_bass_kernel_spmd

    f32 = np.float32
    x_t = np.asarray(x_t, f32)
    h_lstm = np.asarray(h_lstm, f32)
    c_lstm = np.asarray(c_lstm, f32)
    slots = np.asarray(slots, f32)
    ptr_np = np.asarray(ptr)
    ptr_dtype = ptr_np.dtype
    ptr_np = ptr_np.astype(np.int64)

    mem_value_w = np.asarray(mem_value_w, f32)
    mem_value_b = np.asarray(mem_value_b, f32)
    mem_detector_w = np.asarray(mem_detector_w, f32)
    mem_detector_b = np.asarray(mem_detector_b, f32)
    mem_pos_emb = np.asarray(mem_pos_emb, f32)
    mem_weights = np.asarray(mem_weights, f32)
    mem_proj_w = np.asarray(mem_proj_w, f32)
    mem_proj_b = np.asarray(mem_proj_b, f32)
    W_ih_w = np.asarray(W_ih_w, f32)
    W_ih_b = np.asarray(W_ih_b, f32)
    W_hh_w = np.asarray(W_hh_w, f32)

    # ---- host-side weight prep (all tiny, O(weights)) ----
    mw = mem_weights - mem_weights.max()
    ew = np.exp(mw, dtype=f32)
    w = (ew / ew.sum(dtype=f32)).astype(f32)            # softmax(mem_weights)
    posconst = (w[:, None] * mem_pos_emb).sum(0, dtype=f32).astype(f32)  # (D,)
    proj_b_eff = (mem_proj_b + posconst @ mem_proj_w.T).astype(f32)      # (H,)

    # compose the h_mem projection into the gates weights:
    #   gates_hmem = h_mem @ Wg_m.T,  h_mem = fused @ proj_w.T + proj_b_eff
    #   => gates += fused @ (Wg_m @ proj_w).T + Wg_m @ proj_b_eff
    Wg_x = W_ih_w[:, :D]                      # (4H, D)
    Wg_m = W_ih_w[:, D : D + H]               # (4H, H)
    W_comb = (Wg_m @ mem_proj_w).astype(f32)  # (4H, D)
    gbias = (W_ih_b + Wg_m @ proj_b_eff).astype(f32)          # (4H,)
    W_eff = np.concatenate([Wg_x, W_comb, W_hh_w], axis=1)    # (4H, 1536)

    wgT = np.ascontiguousarray(W_eff.T.astype(f32))           # (1536, 2048)
    vwT = np.ascontiguousarray(mem_value_w.T)                 # (D, D)
    dtT = np.ascontiguousarray(mem_detector_w.T)              # (D, 1)
    gbrow = np.ascontiguousarray(gbias.reshape(1, 4 * H))
    has_gbias = bool(np.any(W_ih_b != 0))
    has_vbias = bool(np.any(mem_value_b != 0))

    cfg = {
        "w": tuple(float(x) for x in w),
        "neg_det_b": -float(mem_detector_b.reshape(-1)[0]),
        "has_gbias": has_gbias,
        "has_vbias": has_vbias,
        "mm_f32r": MM_F32R,
    }
    key = (tuple(cfg["w"]), cfg["neg_det_b"], has_gbias, has_vbias, MM_F32R)
    if key not in _NC_CACHE:
        _NC_CACHE[key] = _build_nc(cfg)
    nc = _NC_CACHE[key]

    # ---- shard inputs ----
    in_maps = []
    arange_sl = np.arange(BL, dtype=np.int64) * S
    for m in range(M):
        r0, r1 = m * BL, (m + 1) * BL
        pm = ptr_np[r0:r1]
        xT_m = np.ascontiguousarray(x_t[r0:r1].T)
        im = {
            "xT": xT_m,
            "hT": np.ascontiguousarray(h_lstm[r0:r1].T),
            "c": np.ascontiguousarray(c_lstm[r0:r1]),
            "slots": np.ascontiguousarray(slots[r0:r1].reshape(BL * S, D)),
            "ptrf": pm.astype(f32).reshape(BL, 1),
            "idx": (arange_sl + pm).astype(np.int32).reshape(BL, 1),
            "wptr": w[pm].astype(f32).reshape(BL, 1),
            "wgT": wgT,
            "vwT": vwT,
            "dtT": dtT,
            "gbrow": gbrow,
            "onesrow": np.ones((1, P), dtype=f32),
        }
        if has_vbias:
            im["biasv"] = np.ascontiguousarray(
                np.broadcast_to(mem_value_b[None, :], (P, D))).astype(f32)
        in_maps.append(im)

    res = run_bass_kernel_spmd(nc, in_maps, list(range(M)), trace=TRACE)
    LAST_EXEC_NS = res.exec_time_ns
    LAST_RESULTS = res

    h_new = np.concatenate([r["h_out"] for r in res.results], axis=0)
    c_new = np.concatenate([r["c_out"] for r in res.results], axis=0)
    slots_new = np.concatenate(
        [r["slots_out"].reshape(BL, S, D) for r in res.results], axis=0)
    ptr_new = np.concatenate(
        [r["ptr_out"][:, 0] for r in res.results], axis=0).astype(ptr_dtype)
    return h_new, c_new, slots_new, ptr_new
